# revision 6
# baseline (speedup 1.0000x reference)
"""Self-contained Trainium2 (Bass) kernel for the BaseSigKernel problem.

kernel(xs, ys) -> (24, 24) float32 signature-kernel Gram matrix.

Math (per (x,y) pair; Salvi et al. finite-difference scheme, dyadic_order=1):
    a[r, s]   = <dy[r], dx[s]> / 4          (190x190, dyadic 2x2-duplicated)
    c1 = 1 + a/2 + a^2/12 ;  c2 = 1 - a^2/12
    u[0, :] = u[:, 0] = 1
    u[r+1, s+1] = (u[r+1, s] + u[r, s+1]) * c1[r, s] - u[r, s] * c2[r, s]
    result = u[190, 190]

Distribution: data-parallel over the batch_x axis - core ci owns b in
{3ci, 3ci+1, 3ci+2} x all 24 c's = 72 pairs, held on SBUF partitions
(three 32-partition bands; 24 used per band, the rest compute on zero
padding).

Per core, rows are processed serially; each row is ONE interleaved DVE
tensor_tensor_scan of length 380 alternating
    step 2s  : state = 1     * state + u_prev[s+1]
    step 2s+1: state = c1[s] * state + (-c2[s] * u_prev[s])
which reproduces the reference f32 association (u_left+u_up)*c1 - u_diag*c2
exactly. The scan's data1 is ubuf_prev[3:383] itself: u rows are stored
stride-2 (u[k] at ubuf[2k+1]) and one DVE multiply writes -c2*u into the
dead even lanes. Any reassociation of the per-cell math (e.g. folding the
-c2*u product into scan multipliers via c1/c2 ratios) amplifies ~1000x
through the recurrence and fails the near-zero Gram entries; the exact
association - and hence the per-row TT - is forced.

Measured DVE cost model (TRN2): scan = 153 + 2.08*L ns, tensor_tensor =
155 + 1.04*L ns, independent of partition count and stride. The 2-op row
(TT 190 + scan 380) minimizes fixed+marginal cost; DVE floor is
190*(945+356) = 247us and everything else here is overhead-shaving:

- Coefficients are produced in GROUPS of 8 half-res rows (16 PDE rows)
  with exactly two consumer-visible producer instructions per group (one
  GpSimd even-lane memset + one ScalarE odd-lane Copy-broadcast), so the
  Vector sequencer executes ~2 semaphore waits per 16 rows instead of
  ~2 per 2 rows (each satisfied wait still costs ~68ns of sequencer time).
- Host precomputes group 0 (slots 0-7) in final interleaved form; the
  slot-0 DMA is issued first on the idle SP HWDGE queue so the first scan
  starts ~8.7us instead of ~12.6us (descriptor generation on the Scalar
  queue serializes behind ACT_TABLE_LOAD).
- ub memsets run on GpSimd so the Vector queue's first instruction is the
  row-0 scan.
- The output column (one f32 per partition) is transposed on the idle PE
  via an identity matmul to a contiguous [1, 96] PSUM row before the exit
  DMA: a [96,1] SBUF->DRAM DMA emits 96 4-byte descriptors (~6.8us); the
  transposed form is one descriptor.
"""

import math
from contextlib import ExitStack

import numpy as np

import concourse.bacc as bacc
import concourse.mybir as mybir
import concourse.tile as tile
from concourse.ap import AP

F32 = mybir.dt.float32
Alu = mybir.AluOpType
Act = mybir.ActivationFunctionType

BX, BY, L, DIM = 24, 24, 96, 8
N_CORES = 8
BB = BX // N_CORES          # 3 b-values per core
BAND = 32                   # matmul output base partitions must be 0/32/64
P = BB * BAND               # 96 partitions; 24..31, 56..63, 88..95 are c-padding
NH = L - 1                  # 95: half-resolution grid length
NF = 2 * NH                 # 190: full-resolution grid length
INV_SQRT12 = 1.0 / math.sqrt(12.0)
CF_B = 380                  # coeff slot: [0:380) = [1|c1] interleaved, [380:760) = [x|c2neg] interleaved
W = CF_B + 2 * NF           # 760: coeff slot width
UW = 2 * NF + 4             # u row buffer width (384): u[k] at ubuf[2k+1]
GS = 8                      # coeff slots per production group
RPG = 2 * GS                # 16 PDE rows per group
NG = (NH + GS - 1) // GS    # 12 groups (last has 7 slots)
RING = 3                    # cf group ring


def _view(t_ap: AP, off: int, dims) -> AP:
    """Custom AP view of a tile: dims = [(step, count), ...] incl partition dim."""
    return AP(t_ap.tensor, t_ap.offset + off, [list(d) for d in dims])


def build_bass():
    nc = bacc.Bacc()
    # dyT and dxT packed into one tensor -> one DMA -> one PE sync wait
    inp_d = nc.declare_dram_parameter("inp", [DIM, NH * BAND + BB * NH], F32, isOutput=False)
    cf0_d = nc.declare_dram_parameter("cf0", [P, GS * W], F32, isOutput=False)
    idn_d = nc.declare_dram_parameter("idn", [P, P], F32, isOutput=False)
    out_d = nc.declare_dram_parameter("out", [1, P], F32, isOutput=True)

    with ExitStack() as ctx:
        tc = ctx.enter_context(tile.TileContext(nc))
        sbuf = ctx.enter_context(tc.tile_pool(name="sbuf", bufs=1))
        psum = ctx.enter_context(tc.tile_pool(name="psum", bufs=2, space="PSUM"))
        psum1 = ctx.enter_context(tc.tile_pool(name="psum1", bufs=1, space="PSUM"))

        cfg = [
            sbuf.tile([P, GS * W], F32, name=f"cfg{i}", tag=f"cfg{i}")
            for i in range(RING)
        ]
        inp_t = sbuf.tile([DIM, NH * BAND + BB * NH], F32, name="inp_t", tag="inp_t")
        idn_t = sbuf.tile([P, P], F32, name="idn_t", tag="idn_t")
        ub = [sbuf.tile([P, UW], F32, name=f"u{i}", tag=f"u{i}") for i in range(2)]
        p12g = [
            sbuf.tile([P, GS * 2 * NH], F32, name=f"p12_{i}", tag=f"p12_{i}")
            for i in range(RING)
        ]
        s12g = [
            sbuf.tile([P, GS * NH], F32, name=f"s12{i}", tag=f"s12{i}")
            for i in range(RING)
        ]
        t2g = [
            sbuf.tile([P, GS * NH], F32, name=f"t2{i}", tag=f"t2{i}")
            for i in range(RING)
        ]
        cbias = sbuf.tile([P, 1], F32, name="cbias", tag="cbias")

        # Group 0 (slots 0-7) host-precomputed in final interleaved form.
        # Each consumer deadline gets the earliest possible slot on one of
        # the two HWDGE queues (SP, ACT): slot 0 gates the first scan, slot
        # 1 gates row 2, slots 2-3 row 4, slots 4-7 row 8 (~21us), idn/inp
        # have >5us of slack.
        nc.sync.dma_start(cfg[0][:, 0:W], cf0_d[:, 0:W])
        nc.scalar.dma_start(cfg[0][:, W : 2 * W], cf0_d[:, W : 2 * W])
        nc.scalar.dma_start(cfg[0][:, 2 * W : 4 * W], cf0_d[:, 2 * W : 4 * W])
        nc.sync.dma_start(cfg[0][:, 4 * W : GS * W], cf0_d[:, 4 * W : GS * W])
        nc.scalar.dma_start(inp_t[:], inp_d[:])
        nc.sync.dma_start(idn_t[:], idn_d[:])

        # ub presets on Vector: it idles until the slot-0 DMA lands anyway
        # (GpSimd memsets of this size take ~2.7us and would gate the scan)
        nc.vector.memset(ub[0][:], 1.0)
        nc.vector.memset(ub[1][:], 1.0)
        nc.gpsimd.memset(cbias[:], -1.0)

        def produce_group(g):
            """Slots [8g, 8g+ns) -> cfg[g%RING]; two consumer-visible producers."""
            gi = g % RING
            q0 = g * GS
            ns = min(GS, NH - q0)
            cfgt, p12, s12, t2 = cfg[gi], p12g[gi], s12g[gi], t2g[gi]
            pas = []
            for half in range((ns + 3) // 4):
                lo = half * 4
                hi = min(ns, lo + 4)
                pa_full = psum.tile([P, 512], F32, name=f"pa{g}_{half}", tag=f"pa{half}")
                pas.append(pa_full)
                for j in range(lo, hi):
                    q = q0 + j
                    lhsT = inp_t[:, q * BAND : (q + 1) * BAND]   # [8, 32]
                    for b in range(BB):
                        nc.tensor.matmul(
                            pa_full[b * BAND : (b + 1) * BAND, (j - lo) * NH : (j - lo + 1) * NH],
                            lhsT,
                            inp_t[:, NH * BAND + b * NH : NH * BAND + (b + 1) * NH],
                        )
            for j in range(ns):
                pa = pas[j // 4][:, (j % 4) * NH : (j % 4 + 1) * NH]
                sl = s12[:, j * NH : (j + 1) * NH]
                tl = t2[:, j * NH : (j + 1) * NH]
                # s12 = (a * 1/sqrt(12))^2 = a^2/12
                nc.scalar.activation(sl, pa, Act.Square, scale=INV_SQRT12)
                # t2 = 0.5*a + 1
                nc.scalar.activation(tl, pa, Act.Identity, bias=1.0, scale=0.5)
                # p12 slot layout: [c1h (95) | c2negh (95)]
                nc.scalar.activation(
                    p12[:, j * 2 * NH + NH : (j + 1) * 2 * NH], sl, Act.Identity, bias=cbias[:]
                )
                nc.gpsimd.tensor_tensor(
                    p12[:, j * 2 * NH : j * 2 * NH + NH], tl, sl, Alu.add
                )
            cstep, _ = cfgt.ap[0]
            pstep, _ = p12.ap[0]
            # even lanes (the scan's "1" multipliers) for the whole group
            nc.gpsimd.memset(_view(cfgt, 0, [(cstep, P), (2, ns * CF_B)]), 1.0)
            # odd lanes: the stride-4 pattern runs across slot boundaries, so
            # ONE Copy-broadcast expands all ns slots' c1+c2neg regions.
            nc.scalar.activation(
                _view(cfgt, 1, [(cstep, P), (4, ns * 2 * NH), (2, 2)]),
                _view(p12, 0, [(pstep, P), (1, ns * 2 * NH), (0, 2)]),
                Act.Copy,
            )

        def consume_row(r):
            cfgt = cfg[(r // RPG) % RING]
            off = ((r // 2) % GS) * W
            up = ub[r % 2]
            un = ub[(r + 1) % 2]
            u_step, _ = up.ap[0]
            c_step, _ = cfgt.ap[0]
            if r == 0:
                # u_up == 1: the products are c2neg itself; read data1 straight
                # from the cf slot and skip the TT entirely
                nc.vector.tensor_tensor_scan(
                    un[:, 2 : 2 + 2 * NF],
                    cfgt[:, off : off + 2 * NF],
                    cfgt[:, off + CF_B : off + CF_B + 2 * NF],
                    1.0,
                    Alu.mult,
                    Alu.add,
                )
                return
            # write c2neg[s]*u_prev[s] into the DEAD even lanes of ubuf_prev
            # (they hold last row's scan intermediates), so that
            # ubuf_prev[3:383] is exactly the interleaved scan data1:
            #   t=2s   -> ubuf[3+2s] = u_prev[s+1]
            #   t=2s+1 -> ubuf[4+2s] = c2neg[s]*u_prev[s]
            nc.vector.tensor_tensor(
                _view(up, 4, [(u_step, P), (2, NF)]),
                _view(cfgt, off + CF_B + 1, [(c_step, P), (2, NF)]),
                _view(up, 1, [(u_step, P), (2, NF)]),
                Alu.mult,
            )
            # interleaved scan: state=(d0*state)+d1 over 380 steps
            nc.vector.tensor_tensor_scan(
                un[:, 2 : 2 + 2 * NF],
                cfgt[:, off : off + 2 * NF],
                up[:, 3 : 3 + 2 * NF],
                1.0,
                Alu.mult,
                Alu.add,
            )

        # device production starts at group 1; 2-group lookahead
        produce_group(1)
        produce_group(2)
        for r in range(NF):
            if r % RPG == 0 and RPG <= r <= (NG - 3) * RPG:
                produce_group(r // RPG + 2)
            consume_row(r)

        # transpose the per-partition result column to a contiguous [1, P]
        # PSUM row on the idle PE, bounce through SBUF (DMA cannot read
        # PSUM), then one single-descriptor DMA out
        pout = psum1.tile([BAND, 512], F32, name="pout", tag="pout")
        orow = sbuf.tile([1, P], F32, name="orow", tag="orow")
        nc.tensor.matmul(
            pout[0:1, 0:P], ub[NF % 2][:, 2 * NF + 1 : 2 * NF + 2], idn_t[:, 0:P]
        )
        nc.scalar.activation(orow[0:1, 0:P], pout[0:1, 0:P], Act.Copy)
        nc.sync.dma_start(out_d[:], orow[0:1, 0:P])

    nc.compile()
    return nc


def pack_inputs(xs: np.ndarray, ys: np.ndarray):
    """Full inputs -> per-core in_maps for run_bass_kernel_spmd."""
    xs = np.asarray(xs, np.float32)
    ys = np.asarray(ys, np.float32)
    dx = np.diff(xs, axis=1) * 0.5            # (24, 95, 8)
    dy = np.diff(ys, axis=1) * 0.5            # (24, 95, 8)
    dyT = np.zeros((DIM, NH, BAND), np.float32)
    dyT[:, :, :BY] = dy.transpose(2, 1, 0)
    dyT = dyT.reshape(DIM, NH * BAND)
    inv = np.float32(1.0 / math.sqrt(12.0))
    idn = np.eye(P, dtype=np.float32)
    in_maps = []
    for ci in range(N_CORES):
        dxc = dx[ci * BB : (ci + 1) * BB]     # (3, 95, 8)
        dxT = dxc.transpose(2, 0, 1).reshape(DIM, BB * NH)
        inp = np.ascontiguousarray(np.concatenate([dyT, dxT], axis=1))
        # host-precomputed coeff group 0 (slots 0-7; replicates the device
        # fp32 math - host-vs-PE matmul noise is ~1 ulp and non-systematic)
        cf0 = np.ones((P, GS * W), np.float32)
        rep = np.repeat(np.arange(NH), 2)
        for q in range(GS):
            a0 = np.zeros((P, NH), np.float32)
            for b in range(BB):
                a0[b * BAND : b * BAND + BY] = np.einsum(
                    "cd,jd->cj", dy[:, q, :], dxc[b], dtype=np.float32
                ).astype(np.float32)
            s12 = (a0 * inv) ** 2
            c1 = (np.float32(0.5) * a0 + np.float32(1.0)) + s12
            c2n = s12 - np.float32(1.0)
            cf0[:, q * W + 1 : q * W + 380 : 2] = c1[:, rep]
            cf0[:, q * W + CF_B + 1 : q * W + CF_B + 380 : 2] = c2n[:, rep]
        in_maps.append({"inp": inp, "cf0": cf0, "idn": idn})
    return in_maps


def unpack_outputs(results) -> np.ndarray:
    """Per-core (1,96) outputs -> full (24,24)."""
    out = np.zeros((BX, BY), np.float32)
    for ci in range(N_CORES):
        res = np.asarray(results[ci]["out"]).reshape(P)
        for b in range(BB):
            out[ci * BB + b, :] = res[b * BAND : b * BAND + BY]
    return out


_NC_CACHE = None


def kernel(xs: np.ndarray, ys: np.ndarray) -> np.ndarray:
    """Full (24,96,8) inputs -> full (24,24) output, computed on 8 trn2 cores."""
    global _NC_CACHE
    from concourse.bass_utils import run_bass_kernel_spmd

    if _NC_CACHE is None:
        _NC_CACHE = build_bass()
    in_maps = pack_inputs(xs, ys)
    r = run_bass_kernel_spmd(_NC_CACHE, in_maps, list(range(N_CORES)))
    return unpack_outputs(r.results)


# revision 8
# speedup vs baseline: 1.0241x; 1.0241x over previous
"""Self-contained Trainium2 (Bass) kernel for the BaseSigKernel problem.

kernel(xs, ys) -> (24, 24) float32 signature-kernel Gram matrix.

Math (per (x,y) pair; Salvi et al. finite-difference scheme, dyadic_order=1):
    a[r, s]   = <dy[r], dx[s]> / 4          (190x190, dyadic 2x2-duplicated)
    c1 = 1 + a/2 + a^2/12 ;  c2 = 1 - a^2/12
    u[0, :] = u[:, 0] = 1
    u[r+1, s+1] = (u[r+1, s] + u[r, s+1]) * c1[r, s] - u[r, s] * c2[r, s]
    result = u[190, 190]

Distribution: data-parallel over the batch_x axis - core ci owns b in
{3ci, 3ci+1, 3ci+2} x all 24 c's = 72 pairs, held on SBUF partitions
(three 32-partition bands; 24 used per band, the rest compute on zero
padding).

Per core, rows are processed serially; each row is ONE interleaved DVE
tensor_tensor_scan of length 380 alternating
    step 2s  : state = 1     * state + u_prev[s+1]
    step 2s+1: state = c1[s] * state + (-c2[s] * u_prev[s])
which reproduces the reference f32 association (u_left+u_up)*c1 - u_diag*c2
exactly. The scan's data1 is ubuf_prev[3:383] itself: u rows are stored
stride-2 (u[k] at ubuf[2k+1]) and one DVE multiply writes -c2*u into the
dead even lanes. Any reassociation of the per-cell math (e.g. folding the
-c2*u product into scan multipliers via c1/c2 ratios) amplifies ~1000x
through the recurrence and fails the near-zero Gram entries; the exact
association - and hence the per-row TT - is forced.

Measured DVE cost model (TRN2): scan = 153 + 2.08*L ns, tensor_tensor =
155 + 1.04*L ns, independent of partition count and stride. The 2-op row
(TT 190 + scan 380) minimizes fixed+marginal cost; DVE floor is
190*(945+356) = 247us and everything else here is overhead-shaving:

- Coefficients are produced in GROUPS of 8 half-res rows (16 PDE rows)
  with exactly two consumer-visible producer instructions per group (one
  GpSimd even-lane memset + one ScalarE odd-lane Copy-broadcast), so the
  Vector sequencer executes ~2 semaphore waits per 16 rows instead of
  ~2 per 2 rows (each satisfied wait still costs ~68ns of sequencer time).
- Host precomputes group 0 (slots 0-7) in final interleaved form; the
  slot-0 DMA is issued first on the idle SP HWDGE queue so the first scan
  starts ~8.7us instead of ~12.6us (descriptor generation on the Scalar
  queue serializes behind ACT_TABLE_LOAD).
- ub memsets run on GpSimd so the Vector queue's first instruction is the
  row-0 scan.
- The output column (one f32 per partition) is transposed on the idle PE
  via an identity matmul to a contiguous [1, 96] PSUM row before the exit
  DMA: a [96,1] SBUF->DRAM DMA emits 96 4-byte descriptors (~6.8us); the
  transposed form is one descriptor.
"""

import math
from contextlib import ExitStack

import numpy as np

import concourse.bacc as bacc
import concourse.mybir as mybir
import concourse.tile as tile
from concourse.ap import AP

F32 = mybir.dt.float32
Alu = mybir.AluOpType
Act = mybir.ActivationFunctionType

BX, BY, L, DIM = 24, 24, 96, 8
N_CORES = 8
BB = BX // N_CORES          # 3 b-values per core
BAND = 32                   # matmul output base partitions must be 0/32/64
P = BB * BAND               # 96 partitions; 24..31, 56..63, 88..95 are c-padding
NH = L - 1                  # 95: half-resolution grid length
NF = 2 * NH                 # 190: full-resolution grid length
INV_SQRT12 = 1.0 / math.sqrt(12.0)
CF_B = 380                  # coeff slot: [0:380) = [1|c1] interleaved, [380:760) = [x|c2neg] interleaved
W = CF_B + 2 * NF           # 760: coeff slot width
UW = 2 * NF + 4             # u row buffer width (384): u[k] at ubuf[2k+1]
GS = 8                      # coeff slots per production group
RPG = 2 * GS                # 16 PDE rows per group
NG = (NH + GS - 1) // GS    # 12 groups (last has 7 slots)
RING = 3                    # cf group ring


def _view(t_ap: AP, off: int, dims) -> AP:
    """Custom AP view of a tile: dims = [(step, count), ...] incl partition dim."""
    return AP(t_ap.tensor, t_ap.offset + off, [list(d) for d in dims])


def build_bass():
    nc = bacc.Bacc()
    # dyT and dxT packed into one tensor -> one DMA -> one PE sync wait
    inp_d = nc.declare_dram_parameter("inp", [DIM, NH * BAND + BB * NH], F32, isOutput=False)
    cf0_d = nc.declare_dram_parameter("cf0", [P, GS * W], F32, isOutput=False)
    idn_d = nc.declare_dram_parameter("idn", [P, P], F32, isOutput=False)
    out_d = nc.declare_dram_parameter("out", [1, P], F32, isOutput=True)

    with ExitStack() as ctx:
        tc = ctx.enter_context(tile.TileContext(nc))
        sbuf = ctx.enter_context(tc.tile_pool(name="sbuf", bufs=1))
        psum = ctx.enter_context(tc.tile_pool(name="psum", bufs=2, space="PSUM"))
        psum1 = ctx.enter_context(tc.tile_pool(name="psum1", bufs=1, space="PSUM"))

        cfg = [
            sbuf.tile([P, GS * W], F32, name=f"cfg{i}", tag=f"cfg{i}")
            for i in range(RING)
        ]
        inp_t = sbuf.tile([DIM, NH * BAND + BB * NH], F32, name="inp_t", tag="inp_t")
        idn_t = sbuf.tile([P, P], F32, name="idn_t", tag="idn_t")
        ub = [sbuf.tile([P, UW], F32, name=f"u{i}", tag=f"u{i}") for i in range(2)]
        p12g = [
            sbuf.tile([P, GS * 2 * NH], F32, name=f"p12_{i}", tag=f"p12_{i}")
            for i in range(RING)
        ]
        s12g = [
            sbuf.tile([P, GS * NH], F32, name=f"s12{i}", tag=f"s12{i}")
            for i in range(RING)
        ]
        t2g = [
            sbuf.tile([P, GS * NH], F32, name=f"t2{i}", tag=f"t2{i}")
            for i in range(RING)
        ]
        cbias = sbuf.tile([P, 1], F32, name="cbias", tag="cbias")

        # Group 0 (slots 0-7) host-precomputed in final interleaved form.
        # Keep the DMA count small: more dma_start calls can collapse the
        # transfer onto a single hardware ring (observed: ~3.2us spacing
        # between consecutive DMAs on one queue, and a 7-DMA layout
        # serialized everything through one engine). Slots 0-3 gate rows
        # 0-7 (first deadline ~12.5us), slots 4-7 gate row 8 (~24us), inp
        # gates group-1 production (deadline ~28us), idn the exit (~280us).
        nc.sync.dma_start(cfg[0][:, 0 : 4 * W], cf0_d[:, 0 : 4 * W])
        nc.sync.dma_start(cfg[0][:, 4 * W : GS * W], cf0_d[:, 4 * W : GS * W])
        nc.sync.dma_start(idn_t[:], idn_d[:])
        nc.scalar.dma_start(inp_t[:], inp_d[:])

        # ub presets on Vector: it idles until the slot-0 DMA lands anyway
        # (GpSimd memsets of this size take ~2.7us and would gate the scan)
        nc.vector.memset(ub[0][:], 1.0)
        nc.vector.memset(ub[1][:], 1.0)
        nc.gpsimd.memset(cbias[:], -1.0)

        def produce_group(g):
            """Slots [8g, 8g+ns) -> cfg[g%RING]; two consumer-visible producers."""
            gi = g % RING
            q0 = g * GS
            ns = min(GS, NH - q0)
            cfgt, p12, s12, t2 = cfg[gi], p12g[gi], s12g[gi], t2g[gi]
            pas = []
            for half in range((ns + 3) // 4):
                lo = half * 4
                hi = min(ns, lo + 4)
                pa_full = psum.tile([P, 512], F32, name=f"pa{g}_{half}", tag=f"pa{half}")
                pas.append(pa_full)
                for j in range(lo, hi):
                    q = q0 + j
                    lhsT = inp_t[:, q * BAND : (q + 1) * BAND]   # [8, 32]
                    for b in range(BB):
                        nc.tensor.matmul(
                            pa_full[b * BAND : (b + 1) * BAND, (j - lo) * NH : (j - lo + 1) * NH],
                            lhsT,
                            inp_t[:, NH * BAND + b * NH : NH * BAND + (b + 1) * NH],
                        )
            for j in range(ns):
                pa = pas[j // 4][:, (j % 4) * NH : (j % 4 + 1) * NH]
                sl = s12[:, j * NH : (j + 1) * NH]
                tl = t2[:, j * NH : (j + 1) * NH]
                # s12 = (a * 1/sqrt(12))^2 = a^2/12
                nc.scalar.activation(sl, pa, Act.Square, scale=INV_SQRT12)
                # t2 = 0.5*a + 1
                nc.scalar.activation(tl, pa, Act.Identity, bias=1.0, scale=0.5)
                # p12 slot layout: [c1h (95) | c2negh (95)]
                nc.scalar.activation(
                    p12[:, j * 2 * NH + NH : (j + 1) * 2 * NH], sl, Act.Identity, bias=cbias[:]
                )
                nc.gpsimd.tensor_tensor(
                    p12[:, j * 2 * NH : j * 2 * NH + NH], tl, sl, Alu.add
                )
            cstep, _ = cfgt.ap[0]
            pstep, _ = p12.ap[0]
            # data0 even lanes (the scan's "1" multipliers) are only ever
            # written here and by the group-0 host DMA, and the odd-lane
            # Copy below never touches them - so each ring tile needs this
            # exactly once (groups 1 and 2; tile 0 comes from the host).
            # Skipping it afterwards also keeps the slow Pool engine off
            # the DVE-shared SBUF port during steady state.
            if g < RING:
                nc.gpsimd.memset(
                    _view(cfgt, 0, [(cstep, P), (W, GS), (2, NF)]), 1.0
                )
            # odd lanes: the stride-4 pattern runs across slot boundaries, so
            # ONE Copy-broadcast expands all ns slots' c1+c2neg regions.
            nc.scalar.activation(
                _view(cfgt, 1, [(cstep, P), (4, ns * 2 * NH), (2, 2)]),
                _view(p12, 0, [(pstep, P), (1, ns * 2 * NH), (0, 2)]),
                Act.Copy,
            )

        def consume_row(r):
            cfgt = cfg[(r // RPG) % RING]
            off = ((r // 2) % GS) * W
            up = ub[r % 2]
            un = ub[(r + 1) % 2]
            u_step, _ = up.ap[0]
            c_step, _ = cfgt.ap[0]
            if r == 0:
                # u_up == 1: the products are c2neg itself; read data1 straight
                # from the cf slot and skip the TT entirely
                nc.vector.tensor_tensor_scan(
                    un[:, 2 : 2 + 2 * NF],
                    cfgt[:, off : off + 2 * NF],
                    cfgt[:, off + CF_B : off + CF_B + 2 * NF],
                    1.0,
                    Alu.mult,
                    Alu.add,
                )
                return
            # write c2neg[s]*u_prev[s] into the DEAD even lanes of ubuf_prev
            # (they hold last row's scan intermediates), so that
            # ubuf_prev[3:383] is exactly the interleaved scan data1:
            #   t=2s   -> ubuf[3+2s] = u_prev[s+1]
            #   t=2s+1 -> ubuf[4+2s] = c2neg[s]*u_prev[s]
            nc.vector.tensor_tensor(
                _view(up, 4, [(u_step, P), (2, NF)]),
                _view(cfgt, off + CF_B + 1, [(c_step, P), (2, NF)]),
                _view(up, 1, [(u_step, P), (2, NF)]),
                Alu.mult,
            )
            # interleaved scan: state=(d0*state)+d1 over 380 steps
            nc.vector.tensor_tensor_scan(
                un[:, 2 : 2 + 2 * NF],
                cfgt[:, off : off + 2 * NF],
                up[:, 3 : 3 + 2 * NF],
                1.0,
                Alu.mult,
                Alu.add,
            )

        # device production starts at group 1; 2-group lookahead
        produce_group(1)
        produce_group(2)
        for r in range(NF):
            if r % RPG == 0 and RPG <= r <= (NG - 3) * RPG:
                produce_group(r // RPG + 2)
            consume_row(r)

        # transpose the per-partition result column to a contiguous [1, P]
        # PSUM row on the idle PE, bounce through SBUF (DMA cannot read
        # PSUM), then one single-descriptor DMA out
        pout = psum1.tile([BAND, 512], F32, name="pout", tag="pout")
        orow = sbuf.tile([1, P], F32, name="orow", tag="orow")
        nc.tensor.matmul(
            pout[0:1, 0:P], ub[NF % 2][:, 2 * NF + 1 : 2 * NF + 2], idn_t[:, 0:P]
        )
        nc.scalar.activation(orow[0:1, 0:P], pout[0:1, 0:P], Act.Copy)
        nc.sync.dma_start(out_d[:], orow[0:1, 0:P])

    nc.compile()
    return nc


def pack_inputs(xs: np.ndarray, ys: np.ndarray):
    """Full inputs -> per-core in_maps for run_bass_kernel_spmd."""
    xs = np.asarray(xs, np.float32)
    ys = np.asarray(ys, np.float32)
    dx = np.diff(xs, axis=1) * 0.5            # (24, 95, 8)
    dy = np.diff(ys, axis=1) * 0.5            # (24, 95, 8)
    dyT = np.zeros((DIM, NH, BAND), np.float32)
    dyT[:, :, :BY] = dy.transpose(2, 1, 0)
    dyT = dyT.reshape(DIM, NH * BAND)
    inv = np.float32(1.0 / math.sqrt(12.0))
    idn = np.eye(P, dtype=np.float32)
    in_maps = []
    for ci in range(N_CORES):
        dxc = dx[ci * BB : (ci + 1) * BB]     # (3, 95, 8)
        dxT = dxc.transpose(2, 0, 1).reshape(DIM, BB * NH)
        inp = np.ascontiguousarray(np.concatenate([dyT, dxT], axis=1))
        # host-precomputed coeff group 0 (slots 0-7; replicates the device
        # fp32 math - host-vs-PE matmul noise is ~1 ulp and non-systematic)
        cf0 = np.ones((P, GS * W), np.float32)
        rep = np.repeat(np.arange(NH), 2)
        for q in range(GS):
            a0 = np.zeros((P, NH), np.float32)
            for b in range(BB):
                a0[b * BAND : b * BAND + BY] = np.einsum(
                    "cd,jd->cj", dy[:, q, :], dxc[b], dtype=np.float32
                ).astype(np.float32)
            s12 = (a0 * inv) ** 2
            c1 = (np.float32(0.5) * a0 + np.float32(1.0)) + s12
            c2n = s12 - np.float32(1.0)
            cf0[:, q * W + 1 : q * W + 380 : 2] = c1[:, rep]
            cf0[:, q * W + CF_B + 1 : q * W + CF_B + 380 : 2] = c2n[:, rep]
        in_maps.append({"inp": inp, "cf0": cf0, "idn": idn})
    return in_maps


def unpack_outputs(results) -> np.ndarray:
    """Per-core (1,96) outputs -> full (24,24)."""
    out = np.zeros((BX, BY), np.float32)
    for ci in range(N_CORES):
        res = np.asarray(results[ci]["out"]).reshape(P)
        for b in range(BB):
            out[ci * BB + b, :] = res[b * BAND : b * BAND + BY]
    return out


_NC_CACHE = None


def kernel(xs: np.ndarray, ys: np.ndarray) -> np.ndarray:
    """Full (24,96,8) inputs -> full (24,24) output, computed on 8 trn2 cores."""
    global _NC_CACHE
    from concourse.bass_utils import run_bass_kernel_spmd

    if _NC_CACHE is None:
        _NC_CACHE = build_bass()
    in_maps = pack_inputs(xs, ys)
    r = run_bass_kernel_spmd(_NC_CACHE, in_maps, list(range(N_CORES)))
    return unpack_outputs(r.results)


# revision 11
# speedup vs baseline: 1.0349x; 1.0105x over previous
"""Self-contained Trainium2 (Bass) kernel for the BaseSigKernel problem.

kernel(xs, ys) -> (24, 24) float32 signature-kernel Gram matrix.

Math (per (x,y) pair; Salvi et al. finite-difference scheme, dyadic_order=1):
    a[r, s]   = <dy[r], dx[s]> / 4          (190x190, dyadic 2x2-duplicated)
    c1 = 1 + a/2 + a^2/12 ;  c2 = 1 - a^2/12
    u[0, :] = u[:, 0] = 1
    u[r+1, s+1] = (u[r+1, s] + u[r, s+1]) * c1[r, s] - u[r, s] * c2[r, s]
    result = u[190, 190]

Distribution: data-parallel over the batch_x axis - core ci owns b in
{3ci, 3ci+1, 3ci+2} x all 24 c's = 72 pairs, held on SBUF partitions
(three 32-partition bands; 24 used per band, the rest compute on zero
padding).

Per core, rows are processed serially; each row is ONE interleaved DVE
tensor_tensor_scan of length 380 alternating
    step 2s  : state = 1     * state + u_prev[s+1]
    step 2s+1: state = c1[s] * state + (-c2[s] * u_prev[s])
which reproduces the reference f32 association (u_left+u_up)*c1 - u_diag*c2
exactly. The scan's data1 is ubuf_prev[3:383] itself: u rows are stored
stride-2 (u[k] at ubuf[2k+1]) and one DVE multiply writes -c2*u into the
dead even lanes. Any reassociation of the per-cell math (e.g. folding the
-c2*u product into scan multipliers via c1/c2 ratios) amplifies ~1000x
through the recurrence and fails the near-zero Gram entries; the exact
association - and hence the per-row TT - is forced.

Measured DVE cost model (TRN2): scan = 153 + 2.08*L ns, tensor_tensor =
155 + 1.04*L ns, independent of partition count and stride. The 2-op row
(TT 190 + scan 380) minimizes fixed+marginal cost; DVE floor is
190*(945+356) = 247us and everything else here is overhead-shaving:

- Coefficients are produced in GROUPS of 8 half-res rows (16 PDE rows)
  with exactly two consumer-visible producer instructions per group (one
  GpSimd even-lane memset + one ScalarE odd-lane Copy-broadcast), so the
  Vector sequencer executes ~2 semaphore waits per 16 rows instead of
  ~2 per 2 rows (each satisfied wait still costs ~68ns of sequencer time).
- Host precomputes group 0 (slots 0-7) in final interleaved form; the
  slot-0 DMA is issued first on the idle SP HWDGE queue so the first scan
  starts ~8.7us instead of ~12.6us (descriptor generation on the Scalar
  queue serializes behind ACT_TABLE_LOAD).
- ub memsets run on GpSimd so the Vector queue's first instruction is the
  row-0 scan.
- The output column (one f32 per partition) is transposed on the idle PE
  via an identity matmul to a contiguous [1, 96] PSUM row before the exit
  DMA: a [96,1] SBUF->DRAM DMA emits 96 4-byte descriptors (~6.8us); the
  transposed form is one descriptor.
"""

import math
from contextlib import ExitStack

import numpy as np

import concourse.bacc as bacc
import concourse.mybir as mybir
import concourse.tile as tile
from concourse.ap import AP

F32 = mybir.dt.float32
Alu = mybir.AluOpType
Act = mybir.ActivationFunctionType

BX, BY, L, DIM = 24, 24, 96, 8
N_CORES = 8
BB = BX // N_CORES          # 3 b-values per core
BAND = 32                   # matmul output base partitions must be 0/32/64
P = BB * BAND               # 96 partitions; 24..31, 56..63, 88..95 are c-padding
NH = L - 1                  # 95: half-resolution grid length
NF = 2 * NH                 # 190: full-resolution grid length
INV_SQRT12 = 1.0 / math.sqrt(12.0)
CF_B = 380                  # coeff slot: [0:380) = [1|c1] interleaved, [380:760) = [x|c2neg] interleaved
W = CF_B + 2 * NF           # 760: coeff slot width
UW = 2 * NF + 4             # u row buffer width (384): u[k] at ubuf[2k+1]
GS = 8                      # coeff slots per production group
RPG = 2 * GS                # 16 PDE rows per group
NG = (NH + GS - 1) // GS    # 12 groups (last has 7 slots)
RING = 3                    # cf group ring


def _view(t_ap: AP, off: int, dims) -> AP:
    """Custom AP view of a tile: dims = [(step, count), ...] incl partition dim."""
    return AP(t_ap.tensor, t_ap.offset + off, [list(d) for d in dims])


def build_bass():
    nc = bacc.Bacc()
    # dyT and dxT packed into one tensor -> one DMA -> one PE sync wait
    inp_d = nc.declare_dram_parameter("inp", [DIM, NH * BAND + BB * NH], F32, isOutput=False)
    cf0_d = nc.declare_dram_parameter("cf0", [P, GS * 2 * NH], F32, isOutput=False)
    idn_d = nc.declare_dram_parameter("idn", [P, P], F32, isOutput=False)
    out_d = nc.declare_dram_parameter("out", [1, P], F32, isOutput=True)

    with ExitStack() as ctx:
        tc = ctx.enter_context(tile.TileContext(nc))
        sbuf = ctx.enter_context(tc.tile_pool(name="sbuf", bufs=1))
        psum = ctx.enter_context(tc.tile_pool(name="psum", bufs=2, space="PSUM"))
        psum1 = ctx.enter_context(tc.tile_pool(name="psum1", bufs=1, space="PSUM"))

        cfg = [
            sbuf.tile([P, GS * W], F32, name=f"cfg{i}", tag=f"cfg{i}")
            for i in range(RING)
        ]
        inp_t = sbuf.tile([DIM, NH * BAND + BB * NH], F32, name="inp_t", tag="inp_t")
        idn_t = sbuf.tile([P, P], F32, name="idn_t", tag="idn_t")
        ub = [sbuf.tile([P, UW], F32, name=f"u{i}", tag=f"u{i}") for i in range(2)]
        p12g = [
            sbuf.tile([P, GS * 2 * NH], F32, name=f"p12_{i}", tag=f"p12_{i}")
            for i in range(RING)
        ]
        s12g = [
            sbuf.tile([P, GS * NH], F32, name=f"s12{i}", tag=f"s12{i}")
            for i in range(RING)
        ]
        t2g = [
            sbuf.tile([P, GS * NH], F32, name=f"t2{i}", tag=f"t2{i}")
            for i in range(RING)
        ]
        cbias = sbuf.tile([P, 1], F32, name="cbias", tag="cbias")

        # Group 0 (slots 0-7) arrives host-precomputed in HALF-RES p12 form
        # (583KB instead of 2.33MB: HBM->SBUF runs at only ~270GB/s here,
        # so the full-form payload alone cost ~9us of lead-in) and is
        # expanded on-device by three ScalarE copies, slot 0 first so the
        # row-0 scan's gate is DMA(583KB) + one 190-element copy.
        nc.sync.dma_start(p12g[0][:], cf0_d[:])
        nc.sync.dma_start(idn_t[:], idn_d[:])
        nc.scalar.dma_start(inp_t[:], inp_d[:])

        # Vector idles until the slot-0 expansion lands anyway; use that
        # window for the ub presets and tile-0 even lanes (GpSimd memsets
        # of this size take ~2.7us and would gate the scan).
        nc.vector.memset(ub[0][:], 1.0)
        nc.vector.memset(ub[1][:], 1.0)
        cstep0, _ = cfg[0].ap[0]
        # data0 even lanes ("1" multipliers) for all 8 host slots, plus the
        # data1 even lanes of slot 0 (u_up == 1 for the row-0 scan)
        nc.vector.memset(_view(cfg[0], 0, [(cstep0, P), (W, GS), (2, NF)]), 1.0)
        nc.vector.memset(_view(cfg[0], CF_B, [(cstep0, P), (2, NF)]), 1.0)
        nc.gpsimd.memset(cbias[:], -1.0)

        # expand host p12 -> cfg[0] odd lanes: slot 0 alone (gates scan 0),
        # then slots 1-3 (row-2 deadline), then slots 4-7 (row-8 deadline)
        pstep0, _ = p12g[0].ap[0]
        for lo, hi in ((0, 1), (1, 4), (4, GS)):
            nc.scalar.activation(
                _view(cfg[0], lo * W + 1, [(cstep0, P), (4, (hi - lo) * 2 * NH), (2, 2)]),
                _view(p12g[0], lo * 2 * NH, [(pstep0, P), (1, (hi - lo) * 2 * NH), (0, 2)]),
                Act.Copy,
            )

        def produce_group(g):
            """Slots [8g, 8g+ns) -> cfg[g%RING]; two consumer-visible producers."""
            gi = g % RING
            q0 = g * GS
            ns = min(GS, NH - q0)
            cfgt, p12, s12, t2 = cfg[gi], p12g[gi], s12g[gi], t2g[gi]
            pas = []
            for half in range((ns + 3) // 4):
                lo = half * 4
                hi = min(ns, lo + 4)
                pa_full = psum.tile([P, 512], F32, name=f"pa{g}_{half}", tag=f"pa{half}")
                pas.append(pa_full)
                for j in range(lo, hi):
                    q = q0 + j
                    lhsT = inp_t[:, q * BAND : (q + 1) * BAND]   # [8, 32]
                    for b in range(BB):
                        nc.tensor.matmul(
                            pa_full[b * BAND : (b + 1) * BAND, (j - lo) * NH : (j - lo + 1) * NH],
                            lhsT,
                            inp_t[:, NH * BAND + b * NH : NH * BAND + (b + 1) * NH],
                        )
            for j in range(ns):
                pa = pas[j // 4][:, (j % 4) * NH : (j % 4 + 1) * NH]
                sl = s12[:, j * NH : (j + 1) * NH]
                tl = t2[:, j * NH : (j + 1) * NH]
                # s12 = (a * 1/sqrt(12))^2 = a^2/12
                nc.scalar.activation(sl, pa, Act.Square, scale=INV_SQRT12)
                # t2 = 0.5*a + 1
                nc.scalar.activation(tl, pa, Act.Identity, bias=1.0, scale=0.5)
                # p12 slot layout: [c1h (95) | c2negh (95)]
                nc.scalar.activation(
                    p12[:, j * 2 * NH + NH : (j + 1) * 2 * NH], sl, Act.Identity, bias=cbias[:]
                )
                nc.gpsimd.tensor_tensor(
                    p12[:, j * 2 * NH : j * 2 * NH + NH], tl, sl, Alu.add
                )
            cstep, _ = cfgt.ap[0]
            pstep, _ = p12.ap[0]
            # data0 even lanes (the scan's "1" multipliers) are only ever
            # written here and by the group-0 host DMA, and the odd-lane
            # Copy below never touches them - so each ring tile needs this
            # exactly once (groups 1 and 2; tile 0 comes from the host).
            # Skipping it afterwards also keeps the slow Pool engine off
            # the DVE-shared SBUF port during steady state.
            if g < RING:
                nc.gpsimd.memset(
                    _view(cfgt, 0, [(cstep, P), (W, GS), (2, NF)]), 1.0
                )
            # odd lanes: the stride-4 pattern runs across slot boundaries, so
            # ONE Copy-broadcast expands all ns slots' c1+c2neg regions.
            nc.scalar.activation(
                _view(cfgt, 1, [(cstep, P), (4, ns * 2 * NH), (2, 2)]),
                _view(p12, 0, [(pstep, P), (1, ns * 2 * NH), (0, 2)]),
                Act.Copy,
            )

        def consume_row(r):
            cfgt = cfg[(r // RPG) % RING]
            off = ((r // 2) % GS) * W
            up = ub[r % 2]
            un = ub[(r + 1) % 2]
            u_step, _ = up.ap[0]
            c_step, _ = cfgt.ap[0]
            if r == 0:
                # u_up == 1: the products are c2neg itself; read data1 straight
                # from the cf slot and skip the TT entirely
                nc.vector.tensor_tensor_scan(
                    un[:, 2 : 2 + 2 * NF],
                    cfgt[:, off : off + 2 * NF],
                    cfgt[:, off + CF_B : off + CF_B + 2 * NF],
                    1.0,
                    Alu.mult,
                    Alu.add,
                )
                return
            # write c2neg[s]*u_prev[s] into the DEAD even lanes of ubuf_prev
            # (they hold last row's scan intermediates), so that
            # ubuf_prev[3:383] is exactly the interleaved scan data1:
            #   t=2s   -> ubuf[3+2s] = u_prev[s+1]
            #   t=2s+1 -> ubuf[4+2s] = c2neg[s]*u_prev[s]
            nc.vector.tensor_tensor(
                _view(up, 4, [(u_step, P), (2, NF)]),
                _view(cfgt, off + CF_B + 1, [(c_step, P), (2, NF)]),
                _view(up, 1, [(u_step, P), (2, NF)]),
                Alu.mult,
            )
            # interleaved scan: state=(d0*state)+d1 over 380 steps
            nc.vector.tensor_tensor_scan(
                un[:, 2 : 2 + 2 * NF],
                cfgt[:, off : off + 2 * NF],
                up[:, 3 : 3 + 2 * NF],
                1.0,
                Alu.mult,
                Alu.add,
            )

        # device production starts at group 1; 2-group lookahead
        produce_group(1)
        produce_group(2)
        for r in range(NF):
            if r % RPG == 0 and RPG <= r <= (NG - 3) * RPG:
                produce_group(r // RPG + 2)
            consume_row(r)

        # transpose the per-partition result column to a contiguous [1, P]
        # PSUM row on the idle PE, bounce through SBUF (DMA cannot read
        # PSUM), then one single-descriptor DMA out
        pout = psum1.tile([BAND, 512], F32, name="pout", tag="pout")
        orow = sbuf.tile([1, P], F32, name="orow", tag="orow")
        nc.tensor.matmul(
            pout[0:1, 0:P], ub[NF % 2][:, 2 * NF + 1 : 2 * NF + 2], idn_t[:, 0:P]
        )
        nc.scalar.activation(orow[0:1, 0:P], pout[0:1, 0:P], Act.Copy)
        nc.sync.dma_start(out_d[:], orow[0:1, 0:P])

    nc.compile()
    return nc


def pack_inputs(xs: np.ndarray, ys: np.ndarray):
    """Full inputs -> per-core in_maps for run_bass_kernel_spmd."""
    xs = np.asarray(xs, np.float32)
    ys = np.asarray(ys, np.float32)
    dx = np.diff(xs, axis=1) * 0.5            # (24, 95, 8)
    dy = np.diff(ys, axis=1) * 0.5            # (24, 95, 8)
    dyT = np.zeros((DIM, NH, BAND), np.float32)
    dyT[:, :, :BY] = dy.transpose(2, 1, 0)
    dyT = dyT.reshape(DIM, NH * BAND)
    inv = np.float32(1.0 / math.sqrt(12.0))
    idn = np.eye(P, dtype=np.float32)
    in_maps = []
    for ci in range(N_CORES):
        dxc = dx[ci * BB : (ci + 1) * BB]     # (3, 95, 8)
        dxT = dxc.transpose(2, 0, 1).reshape(DIM, BB * NH)
        inp = np.ascontiguousarray(np.concatenate([dyT, dxT], axis=1))
        # host-precomputed coeff group 0 (slots 0-7) in half-res p12 form
        # [c1h | c2negh] per slot; replicates the device fp32 math -
        # host-vs-PE matmul noise is ~1 ulp and non-systematic
        cf0 = np.zeros((P, GS * 2 * NH), np.float32)
        for q in range(GS):
            a0 = np.zeros((P, NH), np.float32)
            for b in range(BB):
                a0[b * BAND : b * BAND + BY] = np.einsum(
                    "cd,jd->cj", dy[:, q, :], dxc[b], dtype=np.float32
                ).astype(np.float32)
            s12 = (a0 * inv) ** 2
            c1 = (np.float32(0.5) * a0 + np.float32(1.0)) + s12
            c2n = s12 - np.float32(1.0)
            cf0[:, q * 2 * NH : q * 2 * NH + NH] = c1
            cf0[:, q * 2 * NH + NH : (q + 1) * 2 * NH] = c2n
        in_maps.append({"inp": inp, "cf0": cf0, "idn": idn})
    return in_maps


def unpack_outputs(results) -> np.ndarray:
    """Per-core (1,96) outputs -> full (24,24)."""
    out = np.zeros((BX, BY), np.float32)
    for ci in range(N_CORES):
        res = np.asarray(results[ci]["out"]).reshape(P)
        for b in range(BB):
            out[ci * BB + b, :] = res[b * BAND : b * BAND + BY]
    return out


_NC_CACHE = None


def kernel(xs: np.ndarray, ys: np.ndarray) -> np.ndarray:
    """Full (24,96,8) inputs -> full (24,24) output, computed on 8 trn2 cores."""
    global _NC_CACHE
    from concourse.bass_utils import run_bass_kernel_spmd

    if _NC_CACHE is None:
        _NC_CACHE = build_bass()
    in_maps = pack_inputs(xs, ys)
    r = run_bass_kernel_spmd(_NC_CACHE, in_maps, list(range(N_CORES)))
    return unpack_outputs(r.results)


# revision 12
# speedup vs baseline: 1.0362x; 1.0013x over previous
"""Self-contained Trainium2 (Bass) kernel for the BaseSigKernel problem.

kernel(xs, ys) -> (24, 24) float32 signature-kernel Gram matrix.

Math (per (x,y) pair; Salvi et al. finite-difference scheme, dyadic_order=1):
    a[r, s]   = <dy[r], dx[s]> / 4          (190x190, dyadic 2x2-duplicated)
    c1 = 1 + a/2 + a^2/12 ;  c2 = 1 - a^2/12
    u[0, :] = u[:, 0] = 1
    u[r+1, s+1] = (u[r+1, s] + u[r, s+1]) * c1[r, s] - u[r, s] * c2[r, s]
    result = u[190, 190]

Distribution: data-parallel over the batch_x axis - core ci owns b in
{3ci, 3ci+1, 3ci+2} x all 24 c's = 72 pairs, held on SBUF partitions
(three 32-partition bands; 24 used per band, the rest compute on zero
padding).

Per core, rows are processed serially; each row is ONE interleaved DVE
tensor_tensor_scan of length 380 alternating
    step 2s  : state = 1     * state + u_prev[s+1]
    step 2s+1: state = c1[s] * state + (-c2[s] * u_prev[s])
which reproduces the reference f32 association (u_left+u_up)*c1 - u_diag*c2
exactly. The scan's data1 is ubuf_prev[3:383] itself: u rows are stored
stride-2 (u[k] at ubuf[2k+1]) and one DVE multiply writes -c2*u into the
dead even lanes. Any reassociation of the per-cell math (e.g. folding the
-c2*u product into scan multipliers via c1/c2 ratios) amplifies ~1000x
through the recurrence and fails the near-zero Gram entries; the exact
association - and hence the per-row TT - is forced.

Measured DVE cost model (TRN2): scan = 153 + 2.08*L ns, tensor_tensor =
155 + 1.04*L ns, independent of partition count and stride. The 2-op row
(TT 190 + scan 380) minimizes fixed+marginal cost; DVE floor is
190*(945+356) = 247us and everything else here is overhead-shaving:

- Coefficients are produced in GROUPS of 8 half-res rows (16 PDE rows)
  with exactly two consumer-visible producer instructions per group (one
  GpSimd even-lane memset + one ScalarE odd-lane Copy-broadcast), so the
  Vector sequencer executes ~2 semaphore waits per 16 rows instead of
  ~2 per 2 rows (each satisfied wait still costs ~68ns of sequencer time).
- Host precomputes group 0 (slots 0-7) in final interleaved form; the
  slot-0 DMA is issued first on the idle SP HWDGE queue so the first scan
  starts ~8.7us instead of ~12.6us (descriptor generation on the Scalar
  queue serializes behind ACT_TABLE_LOAD).
- ub memsets run on GpSimd so the Vector queue's first instruction is the
  row-0 scan.
- The output column (one f32 per partition) is transposed on the idle PE
  via an identity matmul to a contiguous [1, 96] PSUM row before the exit
  DMA: a [96,1] SBUF->DRAM DMA emits 96 4-byte descriptors (~6.8us); the
  transposed form is one descriptor.
"""

import math
from contextlib import ExitStack

import numpy as np

import concourse.bacc as bacc
import concourse.mybir as mybir
import concourse.tile as tile
from concourse.ap import AP

F32 = mybir.dt.float32
Alu = mybir.AluOpType
Act = mybir.ActivationFunctionType

BX, BY, L, DIM = 24, 24, 96, 8
N_CORES = 8
BB = BX // N_CORES          # 3 b-values per core
BAND = 32                   # matmul output base partitions must be 0/32/64
P = BB * BAND               # 96 partitions; 24..31, 56..63, 88..95 are c-padding
NH = L - 1                  # 95: half-resolution grid length
NF = 2 * NH                 # 190: full-resolution grid length
INV_SQRT12 = 1.0 / math.sqrt(12.0)
CF_B = 380                  # coeff slot: [0:380) = [1|c1] interleaved, [380:760) = [x|c2neg] interleaved
W = CF_B + 2 * NF           # 760: coeff slot width
UW = 2 * NF + 4             # u row buffer width (384): u[k] at ubuf[2k+1]
GS = 8                      # coeff slots per production group
RPG = 2 * GS                # 16 PDE rows per group
NG = (NH + GS - 1) // GS    # 12 groups (last has 7 slots)
RING = 3                    # cf group ring


def _view(t_ap: AP, off: int, dims) -> AP:
    """Custom AP view of a tile: dims = [(step, count), ...] incl partition dim."""
    return AP(t_ap.tensor, t_ap.offset + off, [list(d) for d in dims])


def build_bass():
    nc = bacc.Bacc()
    # dyT and dxT packed into one tensor -> one DMA -> one PE sync wait
    inp_d = nc.declare_dram_parameter("inp", [DIM, NH * BAND + BB * NH], F32, isOutput=False)
    cf0_d = nc.declare_dram_parameter("cf0", [P, GS * 2 * NH], F32, isOutput=False)
    idn_d = nc.declare_dram_parameter("idn", [P, P], F32, isOutput=False)
    out_d = nc.declare_dram_parameter("out", [1, P], F32, isOutput=True)

    with ExitStack() as ctx:
        tc = ctx.enter_context(tile.TileContext(nc))
        sbuf = ctx.enter_context(tc.tile_pool(name="sbuf", bufs=1))
        psum = ctx.enter_context(tc.tile_pool(name="psum", bufs=2, space="PSUM"))
        psum1 = ctx.enter_context(tc.tile_pool(name="psum1", bufs=1, space="PSUM"))

        cfg = [
            sbuf.tile([P, GS * W], F32, name=f"cfg{i}", tag=f"cfg{i}")
            for i in range(RING)
        ]
        inp_t = sbuf.tile([DIM, NH * BAND + BB * NH], F32, name="inp_t", tag="inp_t")
        idn_t = sbuf.tile([P, P], F32, name="idn_t", tag="idn_t")
        ub = [sbuf.tile([P, UW], F32, name=f"u{i}", tag=f"u{i}") for i in range(2)]
        p12g = [
            sbuf.tile([P, GS * 2 * NH], F32, name=f"p12_{i}", tag=f"p12_{i}")
            for i in range(RING)
        ]
        s12g = [
            sbuf.tile([P, GS * NH], F32, name=f"s12{i}", tag=f"s12{i}")
            for i in range(RING)
        ]
        t2g = [
            sbuf.tile([P, GS * NH], F32, name=f"t2{i}", tag=f"t2{i}")
            for i in range(RING)
        ]
        cbias = sbuf.tile([P, 1], F32, name="cbias", tag="cbias")

        # Group 0 (slots 0-7) arrives host-precomputed in HALF-RES p12 form
        # (583KB instead of 2.33MB: HBM->SBUF runs at only ~270GB/s here,
        # so the full-form payload alone cost ~9us of lead-in) and is
        # expanded on-device by three ScalarE copies, slot 0 first so the
        # row-0 scan's gate is DMA(583KB) + one 190-element copy.
        nc.scalar.dma_start(p12g[0][:], cf0_d[:])
        nc.sync.dma_start(inp_t[:], inp_d[:])
        nc.sync.dma_start(idn_t[:], idn_d[:])

        # Vector idles until the slot-0 expansion lands anyway; use that
        # window for the ub presets and tile-0 even lanes (GpSimd memsets
        # of this size take ~2.7us and would gate the scan).
        nc.vector.memset(ub[0][:], 1.0)
        nc.vector.memset(ub[1][:], 1.0)
        cstep0, _ = cfg[0].ap[0]
        # data0 even lanes ("1" multipliers) for all 8 host slots, plus the
        # data1 even lanes of slot 0 (u_up == 1 for the row-0 scan)
        nc.vector.memset(_view(cfg[0], 0, [(cstep0, P), (W, GS), (2, NF)]), 1.0)
        nc.vector.memset(_view(cfg[0], CF_B, [(cstep0, P), (2, NF)]), 1.0)
        nc.gpsimd.memset(cbias[:], -1.0)

        # expand host p12 -> cfg[0] odd lanes: slot 0 alone (gates scan 0),
        # then slots 1-3 (row-2 deadline), then slots 4-7 (row-8 deadline)
        pstep0, _ = p12g[0].ap[0]
        for lo, hi in ((0, 1), (1, 4), (4, GS)):
            nc.scalar.activation(
                _view(cfg[0], lo * W + 1, [(cstep0, P), (4, (hi - lo) * 2 * NH), (2, 2)]),
                _view(p12g[0], lo * 2 * NH, [(pstep0, P), (1, (hi - lo) * 2 * NH), (0, 2)]),
                Act.Copy,
            )

        def produce_group(g):
            """Slots [8g, 8g+ns) -> cfg[g%RING]; two consumer-visible producers."""
            gi = g % RING
            q0 = g * GS
            ns = min(GS, NH - q0)
            cfgt, p12, s12, t2 = cfg[gi], p12g[gi], s12g[gi], t2g[gi]
            pas = []
            for half in range((ns + 3) // 4):
                lo = half * 4
                hi = min(ns, lo + 4)
                pa_full = psum.tile([P, 512], F32, name=f"pa{g}_{half}", tag=f"pa{half}")
                pas.append(pa_full)
                for j in range(lo, hi):
                    q = q0 + j
                    lhsT = inp_t[:, q * BAND : (q + 1) * BAND]   # [8, 32]
                    for b in range(BB):
                        nc.tensor.matmul(
                            pa_full[b * BAND : (b + 1) * BAND, (j - lo) * NH : (j - lo + 1) * NH],
                            lhsT,
                            inp_t[:, NH * BAND + b * NH : NH * BAND + (b + 1) * NH],
                        )
            for j in range(ns):
                pa = pas[j // 4][:, (j % 4) * NH : (j % 4 + 1) * NH]
                sl = s12[:, j * NH : (j + 1) * NH]
                tl = t2[:, j * NH : (j + 1) * NH]
                # s12 = (a * 1/sqrt(12))^2 = a^2/12
                nc.scalar.activation(sl, pa, Act.Square, scale=INV_SQRT12)
                # t2 = 0.5*a + 1
                nc.scalar.activation(tl, pa, Act.Identity, bias=1.0, scale=0.5)
                # p12 slot layout: [c1h (95) | c2negh (95)]
                nc.scalar.activation(
                    p12[:, j * 2 * NH + NH : (j + 1) * 2 * NH], sl, Act.Identity, bias=cbias[:]
                )
                nc.gpsimd.tensor_tensor(
                    p12[:, j * 2 * NH : j * 2 * NH + NH], tl, sl, Alu.add
                )
            cstep, _ = cfgt.ap[0]
            pstep, _ = p12.ap[0]
            # data0 even lanes (the scan's "1" multipliers) are only ever
            # written here and by the group-0 host DMA, and the odd-lane
            # Copy below never touches them - so each ring tile needs this
            # exactly once (groups 1 and 2; tile 0 comes from the host).
            # Skipping it afterwards also keeps the slow Pool engine off
            # the DVE-shared SBUF port during steady state.
            if g < RING:
                nc.gpsimd.memset(
                    _view(cfgt, 0, [(cstep, P), (W, GS), (2, NF)]), 1.0
                )
            # odd lanes: the stride-4 pattern runs across slot boundaries, so
            # ONE Copy-broadcast expands all ns slots' c1+c2neg regions.
            nc.scalar.activation(
                _view(cfgt, 1, [(cstep, P), (4, ns * 2 * NH), (2, 2)]),
                _view(p12, 0, [(pstep, P), (1, ns * 2 * NH), (0, 2)]),
                Act.Copy,
            )

        def consume_row(r):
            cfgt = cfg[(r // RPG) % RING]
            off = ((r // 2) % GS) * W
            up = ub[r % 2]
            un = ub[(r + 1) % 2]
            u_step, _ = up.ap[0]
            c_step, _ = cfgt.ap[0]
            if r == 0:
                # u_up == 1: the products are c2neg itself; read data1 straight
                # from the cf slot and skip the TT entirely
                nc.vector.tensor_tensor_scan(
                    un[:, 2 : 2 + 2 * NF],
                    cfgt[:, off : off + 2 * NF],
                    cfgt[:, off + CF_B : off + CF_B + 2 * NF],
                    1.0,
                    Alu.mult,
                    Alu.add,
                )
                return
            # write c2neg[s]*u_prev[s] into the DEAD even lanes of ubuf_prev
            # (they hold last row's scan intermediates), so that
            # ubuf_prev[3:383] is exactly the interleaved scan data1:
            #   t=2s   -> ubuf[3+2s] = u_prev[s+1]
            #   t=2s+1 -> ubuf[4+2s] = c2neg[s]*u_prev[s]
            nc.vector.tensor_tensor(
                _view(up, 4, [(u_step, P), (2, NF)]),
                _view(cfgt, off + CF_B + 1, [(c_step, P), (2, NF)]),
                _view(up, 1, [(u_step, P), (2, NF)]),
                Alu.mult,
            )
            # interleaved scan: state=(d0*state)+d1 over 380 steps
            nc.vector.tensor_tensor_scan(
                un[:, 2 : 2 + 2 * NF],
                cfgt[:, off : off + 2 * NF],
                up[:, 3 : 3 + 2 * NF],
                1.0,
                Alu.mult,
                Alu.add,
            )

        # device production starts at group 1; 2-group lookahead
        produce_group(1)
        produce_group(2)
        for r in range(NF):
            if r % RPG == 0 and RPG <= r <= (NG - 3) * RPG:
                produce_group(r // RPG + 2)
            consume_row(r)

        # transpose the per-partition result column to a contiguous [1, P]
        # PSUM row on the idle PE, bounce through SBUF (DMA cannot read
        # PSUM), then one single-descriptor DMA out
        pout = psum1.tile([BAND, 512], F32, name="pout", tag="pout")
        orow = sbuf.tile([1, P], F32, name="orow", tag="orow")
        nc.tensor.matmul(
            pout[0:1, 0:P], ub[NF % 2][:, 2 * NF + 1 : 2 * NF + 2], idn_t[:, 0:P]
        )
        nc.scalar.activation(orow[0:1, 0:P], pout[0:1, 0:P], Act.Copy)
        nc.sync.dma_start(out_d[:], orow[0:1, 0:P])

    nc.compile()
    return nc


def pack_inputs(xs: np.ndarray, ys: np.ndarray):
    """Full inputs -> per-core in_maps for run_bass_kernel_spmd."""
    xs = np.asarray(xs, np.float32)
    ys = np.asarray(ys, np.float32)
    dx = np.diff(xs, axis=1) * 0.5            # (24, 95, 8)
    dy = np.diff(ys, axis=1) * 0.5            # (24, 95, 8)
    dyT = np.zeros((DIM, NH, BAND), np.float32)
    dyT[:, :, :BY] = dy.transpose(2, 1, 0)
    dyT = dyT.reshape(DIM, NH * BAND)
    inv = np.float32(1.0 / math.sqrt(12.0))
    idn = np.eye(P, dtype=np.float32)
    in_maps = []
    for ci in range(N_CORES):
        dxc = dx[ci * BB : (ci + 1) * BB]     # (3, 95, 8)
        dxT = dxc.transpose(2, 0, 1).reshape(DIM, BB * NH)
        inp = np.ascontiguousarray(np.concatenate([dyT, dxT], axis=1))
        # host-precomputed coeff group 0 (slots 0-7) in half-res p12 form
        # [c1h | c2negh] per slot; replicates the device fp32 math -
        # host-vs-PE matmul noise is ~1 ulp and non-systematic
        cf0 = np.zeros((P, GS * 2 * NH), np.float32)
        for q in range(GS):
            a0 = np.zeros((P, NH), np.float32)
            for b in range(BB):
                a0[b * BAND : b * BAND + BY] = np.einsum(
                    "cd,jd->cj", dy[:, q, :], dxc[b], dtype=np.float32
                ).astype(np.float32)
            s12 = (a0 * inv) ** 2
            c1 = (np.float32(0.5) * a0 + np.float32(1.0)) + s12
            c2n = s12 - np.float32(1.0)
            cf0[:, q * 2 * NH : q * 2 * NH + NH] = c1
            cf0[:, q * 2 * NH + NH : (q + 1) * 2 * NH] = c2n
        in_maps.append({"inp": inp, "cf0": cf0, "idn": idn})
    return in_maps


def unpack_outputs(results) -> np.ndarray:
    """Per-core (1,96) outputs -> full (24,24)."""
    out = np.zeros((BX, BY), np.float32)
    for ci in range(N_CORES):
        res = np.asarray(results[ci]["out"]).reshape(P)
        for b in range(BB):
            out[ci * BB + b, :] = res[b * BAND : b * BAND + BY]
    return out


_NC_CACHE = None


def kernel(xs: np.ndarray, ys: np.ndarray) -> np.ndarray:
    """Full (24,96,8) inputs -> full (24,24) output, computed on 8 trn2 cores."""
    global _NC_CACHE
    from concourse.bass_utils import run_bass_kernel_spmd

    if _NC_CACHE is None:
        _NC_CACHE = build_bass()
    in_maps = pack_inputs(xs, ys)
    r = run_bass_kernel_spmd(_NC_CACHE, in_maps, list(range(N_CORES)))
    return unpack_outputs(r.results)
